# revision 14
# baseline (speedup 1.0000x reference)
# Binary linear: y[b,s,o] = sum_i x[b,s,i] * sign(W)[o,i]
#
# Strategy (8 NeuronCores, data-parallel over tokens):
#   - Host: flatten x to [32768, 768], shard 8 x [4096, 768], pre-transpose
#     each shard to xT [768, 4096]; split x into fp8 e4m3 parts:
#     hi = e4m3(x) over all 6 contraction blocks, lo = e4m3(x - hi) over
#     the first 4 blocks only. Weights are exactly +-1, so only x carries
#     quantization error; correcting 4/6 of the contraction gives
#     rel err 1.55e-2 (< 2e-2 gate) while cutting PE work to 5/6 of the
#     bf16 stream.
#   - Device (per core): fp8 DoubleRow matmuls — each PE instruction
#     consumes TWO 128-deep contraction tiles (lhsT [p,2,128] stationary,
#     rhs [p,2,N] moving) in N cycles, so the 10 k-tiles (6 hi + 4 lo)
#     per 128-token block cost 5 instructions (vs 6 for bf16):
#       psum[t,o] += sum_j xT_pair[i,j,t].T @ wbinT_pair[i,j,o]
#     PE-stream floor: 32 blocks * 5 * 768 cols = 51.2 us.
#   - y copied out of PSUM as bf16 (halves store traffic), upcast on host.
#   - Host: concat shards -> [4, 8192, 768] f32.

import numpy as np

N_CORES = 8
B, S, D_IN, D_OUT = 4, 8192, 768, 768
T_TOTAL = B * S            # 32768 tokens
T_CORE = T_TOTAL // N_CORES  # 4096 tokens per core
P = 128
IB = D_IN // P             # 6 i-blocks (contraction)
MP = IB // 2               # 3 i-block pairs (DoubleRow)
LB = 4                     # lo-corrected i-blocks (first 4 of 6)
LP = LB // 2               # 2 lo pairs
TB = T_CORE // P           # 32 token-blocks per core
# graduated chunk widths: small first chunks so the PE can start early,
# large later chunks for DMA efficiency.
CHUNKS = [256, 512, 1280, 1024, 1024]
assert sum(CHUNKS) == T_CORE
O_SPLIT = 512              # split for PSUM banks / copies
N_WARMUP = 4               # N=128 warmup matmuls to ramp the HAM clock

_cache = {}


def _build():
    import concourse.bacc as bacc
    import concourse.mybir as mybir
    import concourse.tile as tile

    f32 = mybir.dt.float32
    bf16 = mybir.dt.bfloat16
    fp8 = mybir.dt.float8e4
    DR = mybir.MatmulPerfMode.DoubleRow

    nc = bacc.Bacc(
        "TRN2",
        target_bir_lowering=False,
        debug=False,
        num_devices=N_CORES,
    )

    # x and w arrive pre-packed partition-major (host does the shuffle):
    # each chunk/pair is 128 rows of contiguous bytes, so every load is
    # 128 fat descriptors at DMA line rate (the naive [feature, token]
    # layout would shatter chunk loads into 256-byte descriptors).
    xh = nc.dram_tensor("xh", [P, IB * T_CORE], fp8, kind="ExternalInput")
    xl = nc.dram_tensor("xl", [P, LB * T_CORE], fp8, kind="ExternalInput")
    wT = nc.dram_tensor("wT", [P, IB * D_OUT], fp8, kind="ExternalInput")
    y = nc.dram_tensor("y", [T_CORE, D_OUT], bf16, kind="ExternalOutput")

    with tile.TileContext(nc) as tc:
        with (
            tc.tile_pool(name="wbin", bufs=1) as wbin_pool,
            tc.tile_pool(name="xbuf", bufs=1) as x_pool,
            tc.tile_pool(name="ybuf", bufs=8) as y_pool,
            tc.tile_pool(name="psum", bufs=3, space="PSUM") as psum_pool,
        ):
            chunk_start = []
            s = 0
            for w_ in CHUNKS:
                chunk_start.append(s)
                s += w_

            # --- PE warmup: dummy matmuls on a small zeroed tile during the
            # framework preamble / first DMAs, so the HAM clock gate is at
            # full rate when the real matmuls start. gpsimd memset: the
            # gpsimd queue is idle first (no ACT_TABLE_LOAD / framework
            # preamble congestion), so the warmup starts earliest there. ---
            wu = x_pool.tile([P, P], bf16, tag="warmup", name="wu")
            nc.gpsimd.memset(wu[:], 0.0)
            wups = psum_pool.tile([P, P], f32, tag="wups", name="wups", bufs=1)
            for k in range(N_WARMUP):
                nc.tensor.matmul(
                    wups[:], wu[:], wu[:],
                    start=True, stop=True, skip_group_check=True,
                )
            wu_out = x_pool.tile([P, P], f32, tag="warmup_out", name="wu_out")
            nc.vector.tensor_copy(wu_out[:], wups[:])

            # Pre-binarized (+-1) weight strip PAIRS [128, 2, 768] fp8,
            # spread over the DMA rings so every pair lands early.
            wbin = [None] * MP

            def w_load(m, eng):
                wb = wbin_pool.tile([P, 2, D_OUT], fp8, tag=f"wbin{m}", name=f"wbin{m}")
                eng.dma_start(
                    wb[:],
                    wT[:, 2 * m * D_OUT : (2 * m + 2) * D_OUT].rearrange(
                        "p (b o) -> p b o", b=2
                    ),
                )
                wbin[m] = wb

            xch_h = [None] * len(CHUNKS)
            xch_l = [None] * len(CHUNKS)

            def x_load(c):
                cw = CHUNKS[c]
                c0 = chunk_start[c]
                xth = x_pool.tile([P, IB, cw], fp8, tag=f"xch{c}", name=f"xch{c}")
                nc.sync.dma_start(
                    xth[:],
                    xh[:, IB * c0 : IB * (c0 + cw)].rearrange(
                        "p (b t) -> p b t", b=IB
                    ),
                )
                xch_h[c] = xth
                xtl = x_pool.tile([P, LB, cw], fp8, tag=f"xcl{c}", name=f"xcl{c}")
                nc.sync.dma_start(
                    xtl[:],
                    xl[:, LB * c0 : LB * (c0 + cw)].rearrange(
                        "p (b t) -> p b t", b=LB
                    ),
                )
                xch_l[c] = xtl

            # first wave: wbin0 leads the sync (HWDGE) queue so matmul #0's
            # operands land first; wbin1/wbin2 ride the otherwise-idle
            # gpsimd (SWDGE) and scalar rings.
            w_load(0, nc.sync)
            w_load(1, nc.gpsimd)
            w_load(2, nc.scalar)
            x_load(0)
            x_load(1)
            x_load(2)

            def chunk_of(tok):
                for c in range(len(CHUNKS) - 1, -1, -1):
                    if tok >= chunk_start[c]:
                        return c, tok - chunk_start[c]
                raise AssertionError

            # --- main loop: one 128-token block at a time ---
            for j in range(TB):
                if j == 4:
                    x_load(3)
                elif j == 10:
                    x_load(4)
                c, off = chunk_of(j * P)
                cw = CHUNKS[c]
                ps = psum_pool.tile([P, D_OUT], f32, tag="ps", name=f"ps{j}")
                yt = y_pool.tile([P, D_OUT], bf16, tag="y", name=f"y{j}")
                tail = j >= TB - 2

                # all matmuls of the 512-column half first, then the
                # 256-half: the 512-half PSUM group closes earlier so its
                # (bigger) copy-out overlaps the 256-half matmuls.
                # hi pair 2 goes LAST: its weights ride the scalar ring,
                # which is busy with the framework's ACT_TABLE_LOAD early
                # on — this order buys wbin[2] ~1us of slack at startup.
                SCHED = (
                    (xch_h[c], 0), (xch_h[c], 1),
                    (xch_l[c], 0), (xch_l[c], 1),
                    (xch_h[c], 2),
                )

                def half(lo, hi_):
                    for k, (xc, m) in enumerate(SCHED):
                        lhsT = xc[:, 2 * m : 2 * m + 2, off : off + P]
                        nc.tensor.matmul(
                            ps[:, lo:hi_],
                            lhsT,
                            wbin[m][:, :, lo:hi_],
                            start=(k == 0),
                            stop=(k == len(SCHED) - 1),
                            perf_mode=DR,
                        )

                half(0, O_SPLIT)
                if tail:
                    # tail: the 512-half copy+store launches while the PE
                    # still runs the 256-half matmuls; the (smaller)
                    # 256-half chain rides the ACT engine + its own ring,
                    # so only ~1us of copy/store latency trails the last
                    # matmul.
                    nc.vector.tensor_copy(yt[:, :O_SPLIT], ps[:, :O_SPLIT])
                    nc.sync.dma_start(
                        y[j * P : (j + 1) * P, :O_SPLIT], yt[:, :O_SPLIT]
                    )
                half(O_SPLIT, D_OUT)
                if tail:
                    nc.scalar.copy(yt[:, O_SPLIT:], ps[:, O_SPLIT:])
                    nc.scalar.dma_start(
                        y[j * P : (j + 1) * P, O_SPLIT:], yt[:, O_SPLIT:]
                    )
                else:
                    nc.vector.tensor_copy(yt[:, :O_SPLIT], ps[:, :O_SPLIT])
                    nc.scalar.copy(yt[:, O_SPLIT:], ps[:, O_SPLIT:])
                    eng = nc.sync if j % 2 == 0 else nc.scalar
                    eng.dma_start(y[j * P : (j + 1) * P, :], yt[:])

    nc.compile()
    return nc


def _get_nc():
    if "nc" not in _cache:
        _cache["nc"] = _build()
    return _cache["nc"]


def _pack_chunks(a):
    """[cores, blocks, 128, T] -> [cores, 128, blocks*T] with each token
    chunk's data contiguous per partition row (chunk-major, then
    block-major, token-minor) so chunk loads are 128 fat descriptors."""
    nco, nb, p, t = a.shape
    pieces = []
    s = 0
    for cw in CHUNKS:
        # [cores, nb, 128, cw] -> [cores, 128, nb, cw]
        pieces.append(a[:, :, :, s : s + cw].transpose(0, 2, 1, 3)
                      .reshape(nco, p, nb * cw))
        s += cw
    return np.ascontiguousarray(np.concatenate(pieces, axis=2))


def _prep_inputs(x, weight):
    import ml_dtypes

    f8 = ml_dtypes.float8_e4m3
    x = np.asarray(x, dtype=np.float32)
    w = np.asarray(weight, dtype=np.float32)
    x2 = x.reshape(N_CORES, T_CORE, D_IN)
    # transpose so the contraction dim is on partitions, then hi/lo fp8 split
    xT = np.ascontiguousarray(x2.transpose(0, 2, 1))  # [8, 768, 4096] f32
    xT_hi = xT.astype(f8)
    xT_lo = (
        (xT[:, : LB * P, :] - xT_hi[:, : LB * P, :].astype(np.float32))
    ).astype(f8)
    xh_packed = _pack_chunks(xT_hi.reshape(N_CORES, IB, P, T_CORE))
    xl_packed = _pack_chunks(xT_lo.reshape(N_CORES, LB, P, T_CORE))
    # replicate the small binarized weight: +-1 (and 0) are exact in fp8.
    # pack pair-major: [128, pair, b, o] flattened per partition row.
    wT = np.sign(w).T.astype(f8)  # [i, o]
    w_packed = np.ascontiguousarray(
        wT.reshape(MP, 2, P, D_OUT).transpose(2, 0, 1, 3).reshape(P, IB * D_OUT)
    )
    return [
        {"xh": xh_packed[c], "xl": xl_packed[c], "wT": w_packed}
        for c in range(N_CORES)
    ]


def _install_axon_ntff_hook():
    """The agent image's `antenv` lacks `axon_hooks`; register an equivalent
    module backed by direct ctypes calls into libaxon_pjrt.so so that
    run_bass_kernel_spmd(trace=True) can capture NTFF profiles under axon."""
    import sys

    if "antenv.axon_hooks" in sys.modules:
        return
    import contextlib
    import ctypes
    import types

    so_path = "/opt/axon/libaxon_pjrt.so"
    try:
        lib = ctypes.CDLL(so_path)
    except OSError:
        return
    if not hasattr(lib, "axon_start_nrt_profile"):
        return
    lib.axon_start_nrt_profile.argtypes = [
        ctypes.POINTER(ctypes.c_int64),
        ctypes.c_size_t,
    ]
    lib.axon_start_nrt_profile.restype = ctypes.c_int64
    lib.axon_stop_nrt_profile.argtypes = [ctypes.c_char_p]
    lib.axon_stop_nrt_profile.restype = ctypes.c_int64

    @contextlib.contextmanager
    def _hook(output_dir, device_ids):
        import jax

        jax.devices()
        if device_ids:
            ids = (ctypes.c_int64 * len(device_ids))(*device_ids)
            rc = lib.axon_start_nrt_profile(ids, len(device_ids))
        else:
            rc = lib.axon_start_nrt_profile(None, 0)
        if rc != 0:
            raise RuntimeError(f"axon_start_nrt_profile rc={rc}")
        try:
            yield
        finally:
            n = lib.axon_stop_nrt_profile(str(output_dir).encode())
            print(f"ntff profile: {n} file(s) written to {output_dir}")

    mod = types.ModuleType("antenv.axon_hooks")
    mod.get_axon_ntff_profile_hook = lambda: _hook
    mod.set_axon_ntff_profile_hook = lambda h: None
    sys.modules["antenv.axon_hooks"] = mod


def _run(x, weight, trace=False):
    from concourse.bass_utils import run_bass_kernel_spmd

    if trace:
        _install_axon_ntff_hook()
    nc = _get_nc()
    in_maps = _prep_inputs(x, weight)
    res = run_bass_kernel_spmd(
        nc, in_maps, core_ids=list(range(N_CORES)), trace=trace
    )
    y_full = np.concatenate(
        [r["y"].astype(np.float32) for r in res.results], axis=0
    )
    return y_full.reshape(B, S, D_OUT), res


def kernel(x, weight):
    out, _ = _run(x, weight, trace=False)
    return out
